# revision 11
# baseline (speedup 1.0000x reference)
"""Trainium2 Bass kernel for nn_CategoryInterestAttention.

Contract: kernel(**inputs) takes FULL unsharded inputs (as produced by the
problem's setup_inputs) and returns the FULL (512, 128) float32 output.

Strategy (pure data parallel, 8 NeuronCores, 64 batch rows each):
  - Categories are indexed by VALUE g in [0, 64) instead of the reference's
    sorted-unique slot index; softmax over present groups is permutation
    invariant so the final output is identical.
  - match[g,t] = (cat[t]==g)&mask[t]; query of group g = seq embedding of the
    LAST matching position (argmax of (t+1)*match), gathered by indirect DMA.
  - LayerNorm gains/biases are folded into the projection weights host-side;
    the normalized sequence z=(seq-mu)/sigma is computed once on-chip and
    shared by both layers.
  - bf16 matmuls with fp32 PSUM accumulation; the final target-attention
    stage runs in fp32 (it dominates the error budget).
  - Attention per (row, t-chunk) in a (t, g) layout:
      scoresT = kT_chunk.T @ q_blocked   (q head-block-diagonal => all 4
                                          heads in one matmul)
      E = exp(scoresT*scale) * match     (masking via multiply; exact zeros)
      ctx|den = E_h.T @ [v_h | 1]        (ones column appended to v gives the
                                          softmax denominator for free)
"""

import numpy as np
import ml_dtypes

import concourse.bass as bass
import concourse.bacc as bacc
import concourse.tile as tile
from concourse import mybir
from concourse.bass_utils import run_bass_kernel_spmd

F32 = mybir.dt.float32
BF16 = mybir.dt.bfloat16
I32 = mybir.dt.int32
AF = mybir.ActivationFunctionType
OP = mybir.AluOpType

B, T, D = 512, 256, 128
C, H, L, F = 64, 4, 2, 512
HD = D // H                    # 32
NCORES = 8
R = B // NCORES                # 64 rows per core
NT = R * T                     # 16384 seq tokens per core
NX = R * C                     # 4096 group tokens per core
NTC = NT // 128                # 128 seq chunks
NXT = NX // 128                # 32 x-token tiles
SCALE_S = 1.0 / np.sqrt(np.float32(HD))
SCALE_L = 1.0 / np.sqrt(np.float32(D))
EPS = 1e-5


# ---------------------------------------------------------------------------
# graph build
# ---------------------------------------------------------------------------

def _build(nc):
    seq = nc.dram_tensor("seq", [NT, D], F32, kind="ExternalInput").ap()
    catf = nc.dram_tensor("catf", [R, T], F32, kind="ExternalInput").ap()
    maskf = nc.dram_tensor("maskf", [R, T], F32, kind="ExternalInput").ap()
    tgt = nc.dram_tensor("tgt", [R, D], F32, kind="ExternalInput").ap()
    iota_g2 = nc.dram_tensor("iota_g2", [128, 1], F32, kind="ExternalInput").ap()
    iota_t = nc.dram_tensor("iota_t", [128, T], F32, kind="ExternalInput").ap()
    iota_gr = nc.dram_tensor("iota_gr", [128, C], F32, kind="ExternalInput").ap()
    rowbase = nc.dram_tensor("rowbase", [128, NXT], F32, kind="ExternalInput").ap()
    ident = nc.dram_tensor("ident", [128, 128], F32, kind="ExternalInput").ap()
    io = dict(seq=seq, catf=catf, maskf=maskf, tgt=tgt, iota_g2=iota_g2,
              iota_t=iota_t, iota_gr=iota_gr, rowbase=rowbase, ident=ident)
    for name, shape, dt_ in [
        ("wkt", [D, D], BF16), ("wvt", [D, D], BF16), ("wqt", [D, D], BF16),
        ("wot", [D, D], BF16), ("w1t", [D, F], BF16), ("w2t", [D, F], BF16),
        ("bk", [D, 1], F32), ("bq", [D, 1], F32), ("bo", [D, 1], F32),
        ("b2", [D, 1], F32), ("b1_", [128, 4], F32), ("vb", [128, D], F32),
    ]:
        io[name] = [nc.dram_tensor(f"{name}{l}", shape, dt_,
                                   kind="ExternalInput").ap() for l in range(L)]
    io["out"] = nc.dram_tensor("out", [R, D], F32, kind="ExternalOutput").ap()

    with tile.TileContext(nc) as tc:
        from contextlib import ExitStack
        with ExitStack() as ctx:
            _body(ctx, tc, nc, io)
    return nc


def _body(ctx, tc, nc, io):
    P = 128
    persist = ctx.enter_context(tc.tile_pool(name="persist", bufs=1))
    consts = ctx.enter_context(tc.tile_pool(name="consts", bufs=1))
    ld = ctx.enter_context(tc.tile_pool(name="ld", bufs=4))
    small = ctx.enter_context(tc.tile_pool(name="small", bufs=4))
    ev = ctx.enter_context(tc.tile_pool(name="ev", bufs=4))
    ctokp = ctx.enter_context(tc.tile_pool(name="ctokp", bufs=10))
    epool = ctx.enter_context(tc.tile_pool(name="epool", bufs=6))
    psA = ctx.enter_context(tc.tile_pool(name="psA", bufs=4, space="PSUM"))
    psB = psA
    psC = ctx.enter_context(tc.tile_pool(name="psC", bufs=2, space="PSUM"))
    psX = ctx.enter_context(tc.tile_pool(name="psX", bufs=2, space="PSUM"))

    # ---- constants into SBUF ----
    c_ig2 = consts.tile([P, 1], F32)
    nc.sync.dma_start(out=c_ig2, in_=io["iota_g2"])
    c_it = consts.tile([P, T], F32)
    nc.sync.dma_start(out=c_it, in_=io["iota_t"])
    c_igr = consts.tile([P, C], F32)
    nc.sync.dma_start(out=c_igr, in_=io["iota_gr"])
    c_rb = consts.tile([P, NXT], F32)
    nc.sync.dma_start(out=c_rb, in_=io["rowbase"])
    c_id = consts.tile([P, P], F32)
    nc.sync.dma_start(out=c_id, in_=io["ident"])
    c_eps = consts.tile([P, 1], F32)
    nc.vector.memset(c_eps, EPS)
    w = {}
    for name in ("wkt", "wvt", "wqt", "wot", "w1t", "w2t"):
        w[name] = []
        for l in range(L):
            t_ = consts.tile(list(io[name][l].shape), BF16, tag=f"{name}{l}")
            nc.sync.dma_start(out=t_, in_=io[name][l])
            w[name].append(t_)
    bias = {}
    for name in ("bk", "bq", "bo", "b2", "b1_", "vb"):
        bias[name] = []
        for l in range(L):
            t_ = consts.tile(list(io[name][l].shape), F32, tag=f"{name}{l}")
            nc.sync.dma_start(out=t_, in_=io[name][l])
            bias[name].append(t_)

    # ---- stage 0: z = LN(seq) token-major -> zT feature-major (bf16) ----
    zT = persist.tile([P, NT], BF16)           # (d, 16384)
    cat_tok = persist.tile([P, NTC], F32)      # [t%128, chunk cc=2r+c]
    mask_tok = persist.tile([P, NTC], F32)
    nc.sync.dma_start(out=cat_tok, in_=bass.AP(
        tensor=io["catf"].tensor, offset=0, ap=[[1, 128], [T, R], [128, 2]]))
    nc.sync.dma_start(out=mask_tok, in_=bass.AP(
        tensor=io["maskf"].tensor, offset=0, ap=[[1, 128], [T, R], [128, 2]]))

    for cc in range(NTC):
        s_t = ld.tile([P, D], F32, tag="seqld")
        nc.sync.dma_start(out=s_t, in_=io["seq"][cc * 128:(cc + 1) * 128, :])
        st = small.tile([P, 6], F32, tag="bnst")
        nc.vector.bn_stats(out=st, in_=s_t)
        mv = small.tile([P, 2], F32, tag="bnmv")
        nc.vector.bn_aggr(out=mv, in_=st)
        nc.scalar.activation(out=mv[:, 1:2], in_=mv[:, 1:2], func=AF.Sqrt, bias=c_eps[:])
        nc.vector.reciprocal(out=mv[:, 1:2], in_=mv[:, 1:2])
        z_t = ld.tile([P, D], BF16, tag="ztok")
        nc.vector.tensor_scalar(out=z_t, in0=s_t, scalar1=mv[:, 0:1],
                                scalar2=mv[:, 1:2], op0=OP.subtract, op1=OP.mult)
        nc.sync.dma_start_transpose(out=zT[:, cc * 128:(cc + 1) * 128], in_=z_t)

    # ---- stage 1: qidx / present per row-pair (partition = 64*(r%2)+g) ----
    presentf = persist.tile([P, NXT], F32)     # token-aligned (r,g)
    qposf = persist.tile([P, NXT], F32)
    for rp in range(NXT):
        cat_bc = ld.tile([P, T], F32, tag="catbc")
        msk_bc = ld.tile([P, T], F32, tag="mskbc")
        for half in range(2):
            r = 2 * rp + half
            nc.sync.dma_start(out=cat_bc[64 * half:64 * half + 64, :], in_=bass.AP(
                tensor=io["catf"].tensor, offset=r * T, ap=[[0, 64], [1, T]]))
            nc.sync.dma_start(out=msk_bc[64 * half:64 * half + 64, :], in_=bass.AP(
                tensor=io["maskf"].tensor, offset=r * T, ap=[[0, 64], [1, T]]))
        m_gt = small.tile([P, T], F32, tag="mgt")
        nc.vector.tensor_scalar(out=m_gt, in0=cat_bc, scalar1=c_ig2, scalar2=None,
                                op0=OP.is_equal)
        nc.vector.tensor_tensor(out=m_gt, in0=m_gt, in1=msk_bc, op=OP.mult)
        nc.vector.tensor_reduce(out=presentf[:, rp:rp + 1], in_=m_gt,
                                axis=mybir.AxisListType.X, op=OP.max)
        nc.vector.tensor_tensor(out=m_gt, in0=m_gt, in1=c_it, op=OP.mult)
        nc.vector.tensor_reduce(out=qposf[:, rp:rp + 1], in_=m_gt,
                                axis=mybir.AxisListType.X, op=OP.max)
    # global gather index = max(qpos-1,0) + 256*row ; pen = (present-1)*1e9
    qidx_i = persist.tile([P, NXT], I32)
    tmpq = small.tile([P, NXT], F32, tag="tmpq")
    nc.vector.tensor_scalar(out=tmpq, in0=qposf, scalar1=-1.0, scalar2=0.0,
                            op0=OP.add, op1=OP.max)
    nc.vector.tensor_tensor(out=tmpq, in0=tmpq, in1=c_rb, op=OP.add)
    nc.vector.tensor_copy(out=qidx_i, in_=tmpq)
    pen_tok = persist.tile([P, NXT], F32)
    nc.vector.tensor_scalar(out=pen_tok, in0=presentf, scalar1=-1.0, scalar2=1e9,
                            op0=OP.add, op1=OP.mult)

    # ---- x0 gather (token-major fp32 master copy of x) ----
    x_f = [persist.tile([P, D], F32, tag=f"x{j}", name=f"x{j}")
           for j in range(NXT)]
    for j in range(NXT):
        nc.gpsimd.indirect_dma_start(
            out=x_f[j][:], out_offset=None, in_=io["seq"][:],
            in_offset=bass.IndirectOffsetOnAxis(ap=qidx_i[:, j:j + 1], axis=0))

    # ---- stage 2: attention match tiles (t, g), bf16, layer-shared ----
    m_tg = [persist.tile([P, 1, C], BF16, tag=f"mtg{cc}", name=f"mtg{cc}")
            for cc in range(NTC)]
    for cc in range(NTC):
        nc.vector.scalar_tensor_tensor(
            out=m_tg[cc][:, 0, :], in0=c_igr, scalar=cat_tok[:, cc:cc + 1],
            in1=mask_tok[:, cc:cc + 1].to_broadcast([P, C]),
            op0=OP.is_equal, op1=OP.mult)

    # ---- per-layer persistent buffers ----
    kT = persist.tile([P, NT], BF16)
    v_sb = [persist.tile([P, H, HD + 1], BF16, tag=f"v{cc}", name=f"v{cc}")
            for cc in range(NTC)]
    for cc in range(NTC):
        nc.vector.memset(v_sb[cc][:, :, HD:HD + 1], 1.0)
    xnT = persist.tile([P, NX], BF16)
    q_sb = persist.tile([P, NX], BF16)

    def ln_to(dst_T):
        for j in range(NXT):
            st = small.tile([P, 6], F32, tag="bnst")
            nc.vector.bn_stats(out=st, in_=x_f[j])
            mv = small.tile([P, 2], F32, tag="bnmv")
            nc.vector.bn_aggr(out=mv, in_=st)
            nc.scalar.activation(out=mv[:, 1:2], in_=mv[:, 1:2], func=AF.Sqrt,
                                 bias=c_eps[:])
            nc.vector.reciprocal(out=mv[:, 1:2], in_=mv[:, 1:2])
            zx = ld.tile([P, D], BF16, tag="zxtok")
            nc.vector.tensor_scalar(out=zx, in0=x_f[j], scalar1=mv[:, 0:1],
                                    scalar2=mv[:, 1:2], op0=OP.subtract,
                                    op1=OP.mult)
            nc.sync.dma_start_transpose(out=dst_T[:, j * 128:(j + 1) * 128], in_=zx)

    for l in range(L):
        # ---- kT = Wk' @ z (feature-major) ----
        for nn in range(NT // 512):
            ps = psA.tile([P, 512], F32, tag="mm")
            nc.tensor.matmul(out=ps, lhsT=w["wkt"][l][:],
                             rhs=zT[:, nn * 512:(nn + 1) * 512],
                             start=True, stop=True)
            nc.vector.tensor_scalar(out=kT[:, nn * 512:(nn + 1) * 512], in0=ps,
                                    scalar1=bias["bk"][l][:], scalar2=None,
                                    op0=OP.add)
        # ---- v token-major with ones column (for fused denominators) ----
        for cc in range(NTC):
            ps = psB.tile([P, D], F32, tag="mm")
            nc.tensor.matmul(out=ps, lhsT=zT[:, cc * 128:(cc + 1) * 128],
                             rhs=w["wvt"][l][:], start=True, stop=True)
            nc.vector.tensor_tensor(
                out=v_sb[cc][:, :, 0:HD],
                in0=ps[:].rearrange("p (h d) -> p h d", h=H),
                in1=bias["vb"][l][:].rearrange("p (h d) -> p h d", h=H),
                op=OP.add)
        # ---- q = Wq' @ LN(x) ----
        ln_to(xnT)
        for nn in range(NX // 512):
            ps = psA.tile([P, 512], F32, tag="mm")
            nc.tensor.matmul(out=ps, lhsT=w["wqt"][l][:],
                             rhs=xnT[:, nn * 512:(nn + 1) * 512],
                             start=True, stop=True)
            nc.scalar.activation(out=q_sb[:, nn * 512:(nn + 1) * 512], in_=ps,
                                 func=AF.Identity, bias=bias["bq"][l][:])

        # ---- attention, 8 rows per slab; wo projection per slab ----
        for sl in range(NX // 512):
            ctx_tok = []
            for r in range(8 * sl, 8 * sl + 8):
                qb = epool.tile([P, H * C], BF16, tag="qb")
                nc.vector.memset(qb, 0.0)
                for h in range(H):
                    nc.vector.tensor_copy(
                        out=qb[HD * h:HD * (h + 1), C * h:C * (h + 1)],
                        in_=q_sb[HD * h:HD * (h + 1), C * r:C * (r + 1)])
                e_ch = []
                for c in range(2):
                    cc = 2 * r + c
                    ps = psA.tile([P, H * C], F32, tag="mm")
                    nc.tensor.matmul(out=ps, lhsT=kT[:, cc * 128:(cc + 1) * 128],
                                     rhs=qb, start=True, stop=True)
                    et = epool.tile([P, H, C], BF16, tag="et")
                    nc.scalar.activation(out=et[:], in_=ps[:].rearrange(
                        "p (h g) -> p h g", h=H), func=AF.Exp,
                        scale=float(SCALE_S))
                    E = epool.tile([P, H, C], BF16, tag="E")
                    nc.vector.tensor_tensor(out=E, in0=et,
                                            in1=m_tg[cc].to_broadcast([P, H, C]),
                                            op=OP.mult)
                    e_ch.append(E)
                psc = psC.tile([C, H, HD + 1], F32, tag="psctx")
                first = True
                for h in range(H):
                    for c in range(2):
                        nc.tensor.matmul(
                            out=psc[:, h, :], lhsT=e_ch[c][:, h, :],
                            rhs=v_sb[2 * r + c][:, h, :],
                            start=first, stop=(h == H - 1 and c == 1))
                        first = False
                rd = small.tile([C, H, 1], F32, tag="rd")
                nc.vector.tensor_scalar(out=rd, in0=psc[:, :, HD:HD + 1],
                                        scalar1=1e-30, scalar2=None, op0=OP.add)
                nc.vector.reciprocal(out=rd, in_=rd)
                ct = ctokp.tile([C, D], BF16, tag="ctok")
                nc.vector.scalar_tensor_tensor(
                    out=ct[:].rearrange("p (h d) -> p h d", h=H),
                    in0=psc[:, :, 0:HD], scalar=1.0,
                    in1=rd.to_broadcast([C, H, HD]),
                    op0=OP.mult, op1=OP.mult)
                ctx_tok.append(ct)
            cT = ev.tile([P, 512], BF16, tag="cT")
            for k in range(8):
                nc.sync.dma_start_transpose(
                    out=cT[:, k * 64:(k + 1) * 64], in_=ctx_tok[k])
            ps = psA.tile([P, 512], F32, tag="mm")
            nc.tensor.matmul(out=ps, lhsT=w["wot"][l][:], rhs=cT,
                             start=True, stop=True)
            aoT = ev.tile([P, 512], BF16, tag="aoT")
            nc.vector.tensor_scalar(out=aoT, in0=ps, scalar1=bias["bo"][l][:],
                                    scalar2=None, op0=OP.add)
            for k in range(4):
                j = sl * 4 + k
                at = ev.tile([P, D], BF16, tag="atok")
                nc.sync.dma_start_transpose(out=at, in_=aoT[:, k * 128:(k + 1) * 128])
                nc.vector.tensor_tensor(out=x_f[j], in0=x_f[j], in1=at, op=OP.add)

        # ---- FFN ----
        ln_to(xnT)
        for nn in range(NX // 512):
            r1 = []
            for fc in range(4):
                ps = psA.tile([P, 512], F32, tag="mm")
                nc.tensor.matmul(out=ps,
                                 lhsT=w["w1t"][l][:, fc * 128:(fc + 1) * 128],
                                 rhs=xnT[:, nn * 512:(nn + 1) * 512],
                                 start=True, stop=True)
                r1t = ev.tile([P, 512], BF16, tag="r1")
                nc.vector.tensor_scalar(out=r1t, in0=ps,
                                        scalar1=bias["b1_"][l][:, fc:fc + 1],
                                        scalar2=0.0, op0=OP.add, op1=OP.max)
                r1.append(r1t)
            ps2 = psB.tile([P, 512], F32, tag="mm")
            for fc in range(4):
                nc.tensor.matmul(out=ps2,
                                 lhsT=w["w2t"][l][:, fc * 128:(fc + 1) * 128],
                                 rhs=r1[fc], start=(fc == 0), stop=(fc == 3))
            f2T = ev.tile([P, 512], BF16, tag="aoT")
            nc.vector.tensor_scalar(out=f2T, in0=ps2, scalar1=bias["b2"][l][:],
                                    scalar2=None, op0=OP.add)
            for k in range(4):
                j = nn * 4 + k
                ft = ev.tile([P, D], BF16, tag="atok")
                nc.sync.dma_start_transpose(out=ft, in_=f2T[:, k * 128:(k + 1) * 128])
                nc.vector.tensor_tensor(out=x_f[j], in0=x_f[j], in1=ft, op=OP.add)

    # ---- final stage (fp32): logits, softmax over groups, weighted sum ----
    Lgr = persist.tile([P, R], F32)            # [64*(r%2)+g, r]
    nc.vector.memset(Lgr, -1e9)
    scratch = small.tile([P, D], F32, tag="fsc")
    for r in range(R):
        off = 64 * (r % 2)
        tb = ld.tile([P, D], F32, tag="tgtbc")
        nc.sync.dma_start(out=tb[off:off + 64, :], in_=bass.AP(
            tensor=io["tgt"].tensor, offset=r * D, ap=[[0, 64], [1, D]]))
        nc.vector.scalar_tensor_tensor(
            out=scratch[off:off + 64, :], in0=x_f[r // 2][off:off + 64, :],
            scalar=float(SCALE_L), in1=tb[off:off + 64, :],
            op0=OP.mult, op1=OP.mult)
        nc.vector.tensor_reduce(out=Lgr[off:off + 64, r:r + 1],
                                in_=scratch[off:off + 64, :],
                                axis=mybir.AxisListType.X, op=OP.add)
    # presence penalty: even rows live in [0:64, even cols], odd in [64:128, odd]
    for par in range(2):
        lp = Lgr[64 * par:64 * par + 64, :].rearrange("p (j two) -> p j two", two=2)
        nc.vector.tensor_tensor(
            out=lp[:, :, par:par + 1], in0=lp[:, :, par:par + 1],
            in1=pen_tok[64 * par:64 * par + 64, :].rearrange("p (j o) -> p j o", o=1),
            op=OP.add)
    psL = psX.tile([R, P], F32, tag="aux")
    nc.tensor.transpose(out=psL, in_=Lgr, identity=c_id[:])
    Erg = persist.tile([R, P], F32)
    den = small.tile([R, 1], F32, tag="den")
    nc.scalar.activation(out=Erg, in_=psL, func=AF.Exp, accum_out=den)
    nc.vector.reciprocal(out=den, in_=den)
    nc.vector.tensor_scalar(out=Erg, in0=Erg, scalar1=den, scalar2=None,
                            op0=OP.mult)
    psW = psX.tile([P, R], F32, tag="aux")
    nc.tensor.transpose(out=psW, in_=Erg, identity=c_id[0:R, 0:R])
    wT = persist.tile([P, R], F32)
    nc.vector.tensor_copy(out=wT, in_=psW)
    # wT columns have exact zeros in the off-parity half (exp(-1e9) == 0), so
    # a (128,2) stationary against the full token tile gives both rows at once.
    for j in range(NXT):
        psO = psX.tile([2, D], F32, tag="aux")
        nc.tensor.matmul(out=psO, lhsT=wT[:, 2 * j:2 * j + 2],
                         rhs=x_f[j][:], start=True, stop=True)
        o_sb = ev.tile([2, D], F32, tag="osb")
        nc.vector.tensor_copy(out=o_sb, in_=psO)
        nc.sync.dma_start(out=io["out"][2 * j:2 * j + 2, :], in_=o_sb)


# ---------------------------------------------------------------------------
# host side
# ---------------------------------------------------------------------------

_NC_CACHE = {}


def _get_nc():
    if "nc" not in _NC_CACHE:
        nc = bacc.Bacc("TRN2", target_bir_lowering=False, debug=False,
                       enable_asserts=False)
        _build(nc)
        nc.compile()
        _NC_CACHE["nc"] = nc
    return _NC_CACHE["nc"]


def _consts():
    p = np.arange(128)
    iota_g2 = (p % 64).astype(np.float32)[:, None]
    iota_t = np.tile((np.arange(T) + 1.0).astype(np.float32), (128, 1))
    iota_gr = np.tile(np.arange(C, dtype=np.float32), (128, 1))
    col = np.arange(NXT)
    rowbase = (256.0 * (2 * col[None, :] + p[:, None] // 64)).astype(np.float32)
    ident = np.eye(128, dtype=np.float32)
    return dict(iota_g2=iota_g2, iota_t=iota_t, iota_gr=iota_gr,
                rowbase=rowbase, ident=ident)


def _prep_weights(inp):
    wqkv = np.asarray(inp["wqkv"], np.float32)
    bqkv = np.asarray(inp["bqkv"], np.float32)
    wo = np.asarray(inp["wo"], np.float32)
    bo = np.asarray(inp["bo"], np.float32)
    l1g = np.asarray(inp["ln1_g"], np.float32)
    l1b = np.asarray(inp["ln1_b"], np.float32)
    l2g = np.asarray(inp["ln2_g"], np.float32)
    l2b = np.asarray(inp["ln2_b"], np.float32)
    w1 = np.asarray(inp["w1"], np.float32)
    b1 = np.asarray(inp["b1"], np.float32)
    w2 = np.asarray(inp["w2"], np.float32)
    b2 = np.asarray(inp["b2"], np.float32)
    Wq, Wk, Wv = wqkv[:, :D], wqkv[:, D:2 * D], wqkv[:, 2 * D:]
    bq_, bk_, bv_ = bqkv[:, :D], bqkv[:, D:2 * D], bqkv[:, 2 * D:]
    bf = lambda x: np.ascontiguousarray(x.astype(ml_dtypes.bfloat16))
    f32 = lambda x: np.ascontiguousarray(x.astype(np.float32))
    m = {}
    for l in range(L):
        Wqp = Wq[l] * l1g[l][None, :]
        Wkp = Wk[l] * l1g[l][None, :]
        Wvp = Wv[l] * l1g[l][None, :]
        W1p = w1[l] * l2g[l][None, :]
        bqp = Wq[l] @ l1b[l] + bq_[l]
        bkp = Wk[l] @ l1b[l] + bk_[l]
        bvp = Wv[l] @ l1b[l] + bv_[l]
        b1p = w1[l] @ l2b[l] + b1[l]
        m[f"wkt{l}"] = bf(Wkp.T)              # (d, d')
        m[f"wvt{l}"] = bf(Wvp.T)
        m[f"wqt{l}"] = bf(Wqp.T)
        m[f"wot{l}"] = bf(wo[l].T)            # (d, d_out)
        m[f"w1t{l}"] = bf(W1p.T)              # (d, f)
        w2tl = np.empty((128, F), np.float32)  # w2t[p, 128*fc+d] = W2[d, 128*fc+p]
        for fc in range(4):
            w2tl[:, fc * 128:(fc + 1) * 128] = w2[l][:, fc * 128:(fc + 1) * 128].T
        m[f"w2t{l}"] = bf(w2tl)
        m[f"bk{l}"] = f32(bkp[:, None])
        m[f"bq{l}"] = f32(bqp[:, None])
        m[f"bo{l}"] = f32(bo[l][:, None])
        m[f"b2{l}"] = f32(b2[l][:, None])
        m[f"b1_{l}"] = f32(b1p.reshape(4, 128).T)
        m[f"vb{l}"] = f32(np.tile(bvp[None, :], (128, 1)))
    return m


def kernel(**inputs):
    nc = _get_nc()
    wm = _prep_weights(inputs)
    cm = _consts()
    seq = np.asarray(inputs["sequence_item_emb"], np.float32)
    cat = np.asarray(inputs["sequence_cat_ids"])
    msk = np.asarray(inputs["sequence_mask"])
    tgt = np.asarray(inputs["target_item_emb"], np.float32)
    in_maps = []
    for i in range(NCORES):
        rs = slice(i * R, (i + 1) * R)
        im = dict(wm)
        im.update(cm)
        im["seq"] = np.ascontiguousarray(seq[rs].reshape(NT, D))
        im["catf"] = np.ascontiguousarray(cat[rs].astype(np.float32))
        im["maskf"] = np.ascontiguousarray(msk[rs].astype(np.float32))
        im["tgt"] = np.ascontiguousarray(tgt[rs])
        in_maps.append(im)
    res = run_bass_kernel_spmd(nc, in_maps, list(range(NCORES)))
    _NC_CACHE["last"] = res
    return np.concatenate([res.results[i]["out"] for i in range(NCORES)], axis=0)


# revision 17
# speedup vs baseline: 1.9507x; 1.9507x over previous
"""Trainium2 Bass kernel for nn_CategoryInterestAttention.

Contract: kernel(**inputs) takes FULL unsharded inputs (as produced by the
problem's setup_inputs) and returns the FULL (512, 128) float32 output.

Strategy (pure data parallel, 8 NeuronCores, 64 batch rows each):
  - Categories are indexed by VALUE g in [0, 64) instead of the reference's
    sorted-unique slot index; softmax over present groups is permutation
    invariant so the final output is identical.
  - match[g,t] = (cat[t]==g)&mask[t]; query of group g = seq embedding of the
    LAST matching position (argmax of (t+1)*match), gathered by indirect DMA.
  - LayerNorm gains/biases are folded into the projection weights host-side;
    the normalized sequence z=(seq-mu)/sigma is computed once on-chip and
    shared by both layers. The v bias is folded into the wo bias (attention
    weights sum to 1 per group).
  - bf16 matmuls with fp32 PSUM accumulation; the final target-attention
    stage runs in fp32 (it dominates the error budget).
  - Attention per (row, t-chunk) in a (t, g) layout:
      scoresT  = kT_chunk.T @ q_blocked + (BIG/s)*match  (two matmuls into
                 one PSUM tile; q head-block-diagonal => 4 heads at once)
      E        = exp(s*scoresT - BIG)     (masking folded into the exp bias;
                                           non-matching entries ~1e-13)
      ctx|den  = E_hh.T @ [v_h0|1|v_h1|1] (ones columns give the softmax
                                           denominators for free)
  - All transposes on the PE (DMA transpose measured ~1.2us each on the sync
    engine); attention/FFN outputs are transposed into PSUM and added to the
    fp32 token-major residual directly from PSUM.
"""

import numpy as np
import ml_dtypes

import concourse.bass as bass
import concourse.bacc as bacc
import concourse.tile as tile
from concourse import mybir
from concourse.bass_utils import run_bass_kernel_spmd

F32 = mybir.dt.float32
BF16 = mybir.dt.bfloat16
I32 = mybir.dt.int32
AF = mybir.ActivationFunctionType
OP = mybir.AluOpType

B, T, D = 512, 256, 128
C, H, L, F = 64, 4, 2, 512
HD = D // H                    # 32
NCORES = 8
R = B // NCORES                # 64 rows per core
NT = R * T                     # 16384 seq tokens per core
NX = R * C                     # 4096 group tokens per core
NTC = NT // 128                # 128 seq chunks
NXT = NX // 128                # 32 x-token tiles
SCALE_S = 1.0 / np.sqrt(np.float32(HD))
SCALE_L = 1.0 / np.sqrt(np.float32(D))
EPS = 1e-5
BIG = 30.0                     # mask offset: exp(-30) ~ 9e-14


def _build(nc):
    seq = nc.dram_tensor("seq", [NT, D], F32, kind="ExternalInput").ap()
    catf = nc.dram_tensor("catf", [R, T], F32, kind="ExternalInput").ap()
    maskf = nc.dram_tensor("maskf", [R, T], F32, kind="ExternalInput").ap()
    tgt = nc.dram_tensor("tgt", [R, D], F32, kind="ExternalInput").ap()
    iota_g2 = nc.dram_tensor("iota_g2", [128, 1], F32, kind="ExternalInput").ap()
    iota_t = nc.dram_tensor("iota_t", [128, T], F32, kind="ExternalInput").ap()
    rowbase = nc.dram_tensor("rowbase", [128, NXT], F32, kind="ExternalInput").ap()
    ident = nc.dram_tensor("ident", [128, 128], F32, kind="ExternalInput").ap()
    identb = nc.dram_tensor("identb", [128, 128], BF16, kind="ExternalInput").ap()
    nbeye = nc.dram_tensor("nbeye", [2 * C, H * C], BF16, kind="ExternalInput").ap()
    io = dict(seq=seq, catf=catf, maskf=maskf, tgt=tgt, iota_g2=iota_g2,
              iota_t=iota_t, rowbase=rowbase, ident=ident, identb=identb,
              nbeye=nbeye)
    for name, shape, dt_ in [
        ("wkt", [D, D], BF16), ("wvt", [D, D], BF16), ("wqt", [D, D], BF16),
        ("wot", [D, D], BF16), ("w1t", [D, F], BF16), ("w2t", [D, F], BF16),
        ("bk", [D, 1], F32), ("bq", [D, 1], F32), ("bo", [D, 1], F32),
        ("b2", [D, 1], F32), ("b1_", [128, 4], F32),
    ]:
        io[name] = [nc.dram_tensor(f"{name}{l}", shape, dt_,
                                   kind="ExternalInput").ap() for l in range(L)]
    io["out"] = nc.dram_tensor("out", [R, D], F32, kind="ExternalOutput").ap()

    with tile.TileContext(nc) as tc:
        from contextlib import ExitStack
        with ExitStack() as ctx:
            _body(ctx, tc, nc, io)
    return nc


def _body(ctx, tc, nc, io):
    P = 128
    persist = ctx.enter_context(tc.tile_pool(name="persist", bufs=1))
    consts = ctx.enter_context(tc.tile_pool(name="consts", bufs=1))
    ld = ctx.enter_context(tc.tile_pool(name="ld", bufs=4))
    small = ctx.enter_context(tc.tile_pool(name="small", bufs=4))
    ev = ctx.enter_context(tc.tile_pool(name="ev", bufs=4))
    ctokp = ctx.enter_context(tc.tile_pool(name="ctokp", bufs=6))
    epool = ctx.enter_context(tc.tile_pool(name="epool", bufs=6))
    psA = ctx.enter_context(tc.tile_pool(name="psA", bufs=3, space="PSUM"))
    psC = ctx.enter_context(tc.tile_pool(name="psC", bufs=2, space="PSUM"))
    psT = ctx.enter_context(tc.tile_pool(name="psT", bufs=2, space="PSUM"))
    psX = ctx.enter_context(tc.tile_pool(name="psX", bufs=1, space="PSUM"))

    # ---- constants ----
    c_ig2 = consts.tile([P, 1], F32)
    nc.sync.dma_start(out=c_ig2, in_=io["iota_g2"])
    c_it = consts.tile([P, T], F32)
    nc.sync.dma_start(out=c_it, in_=io["iota_t"])
    c_rb = consts.tile([P, NXT], F32)
    nc.sync.dma_start(out=c_rb, in_=io["rowbase"])
    c_id = consts.tile([P, P], F32)
    nc.sync.dma_start(out=c_id, in_=io["ident"])
    c_idb = consts.tile([P, P], BF16)
    nc.sync.dma_start(out=c_idb, in_=io["identb"])
    c_nbe = consts.tile([2 * C, H * C], BF16)
    nc.sync.dma_start(out=c_nbe, in_=io["nbeye"])
    c_eps = consts.tile([P, 1], F32)
    nc.vector.memset(c_eps, EPS)
    c_nbig = consts.tile([P, 1], F32)
    nc.vector.memset(c_nbig, -BIG)
    w = {}
    for name in ("wkt", "wvt", "wqt", "wot", "w1t", "w2t"):
        w[name] = []
        for l in range(L):
            t_ = consts.tile(list(io[name][l].shape), BF16, tag=f"{name}{l}")
            nc.sync.dma_start(out=t_, in_=io[name][l])
            w[name].append(t_)
    bias = {}
    for name in ("bk", "bq", "bo", "b2", "b1_"):
        bias[name] = []
        for l in range(L):
            t_ = consts.tile(list(io[name][l].shape), F32, tag=f"{name}{l}")
            nc.sync.dma_start(out=t_, in_=io[name][l])
            bias[name].append(t_)

    def pe_transpose_b(dst_sbuf_slice, src_tile):
        """bf16 (128,128) transpose via PE + ACT evict into an SBUF slice."""
        pt = psT.tile([P, P], BF16, tag="pt")
        nc.tensor.transpose(out=pt, in_=src_tile, identity=c_idb[:])
        nc.scalar.copy(out=dst_sbuf_slice, in_=pt)

    # ---- stage 0: z = LN(seq) token-major -> zT feature-major (bf16) ----
    zT = persist.tile([P, NT], BF16)
    for g4 in range(NT // 512):
        s4 = ld.tile([P, 512], F32, tag="seqld")
        nc.sync.dma_start(out=s4, in_=bass.AP(
            tensor=io["seq"].tensor, offset=g4 * 512 * D,
            ap=[[D, 128], [128 * D, 4], [1, D]]))
        mvb = small.tile([P, 4, 2], F32, tag="mvb")
        for j in range(4):
            st = small.tile([P, 6], F32, tag="bnst")
            nc.vector.bn_stats(out=st, in_=s4[:, j * 128:(j + 1) * 128])
            nc.vector.bn_aggr(out=mvb[:, j, :], in_=st)
        nc.scalar.activation(out=mvb[:, :, 1:2], in_=mvb[:, :, 1:2],
                             func=AF.Sqrt, bias=c_eps[:])
        nc.vector.reciprocal(out=mvb[:, :, 1:2], in_=mvb[:, :, 1:2])
        z4 = ld.tile([P, 512], BF16, tag="ztok")
        for j in range(4):
            nc.vector.tensor_scalar(
                out=z4[:, j * 128:(j + 1) * 128],
                in0=s4[:, j * 128:(j + 1) * 128], scalar1=mvb[:, j, 0:1],
                scalar2=mvb[:, j, 1:2], op0=OP.subtract, op1=OP.mult)
            pe_transpose_b(zT[:, g4 * 512 + j * 128:g4 * 512 + (j + 1) * 128],
                           z4[:, j * 128:(j + 1) * 128])

    # ---- stage 1: match (g-layout), qidx, present per row-pair ----
    m_gt = [persist.tile([P, T], BF16, tag=f"mgt{rp}", name=f"mgt{rp}")
            for rp in range(NXT)]
    presentf = persist.tile([P, NXT], F32)
    qposf = persist.tile([P, NXT], F32)
    for rp in range(NXT):
        cat_bc = ld.tile([P, T], F32, tag="catbc")
        msk_bc = ld.tile([P, T], F32, tag="mskbc")
        for half in range(2):
            r = 2 * rp + half
            nc.gpsimd.dma_start(out=cat_bc[64 * half:64 * half + 64, :],
                                in_=bass.AP(tensor=io["catf"].tensor,
                                            offset=r * T, ap=[[0, 64], [1, T]]))
            nc.gpsimd.dma_start(out=msk_bc[64 * half:64 * half + 64, :],
                                in_=bass.AP(tensor=io["maskf"].tensor,
                                            offset=r * T, ap=[[0, 64], [1, T]]))
        nc.vector.scalar_tensor_tensor(out=m_gt[rp], in0=cat_bc, scalar=c_ig2,
                                       in1=msk_bc, op0=OP.is_equal, op1=OP.mult)
        nc.vector.tensor_reduce(out=presentf[:, rp:rp + 1], in_=m_gt[rp],
                                axis=mybir.AxisListType.X, op=OP.max)
        posm = small.tile([P, T], F32, tag="posm")
        nc.vector.tensor_tensor(out=posm, in0=m_gt[rp], in1=c_it, op=OP.mult)
        nc.vector.tensor_reduce(out=qposf[:, rp:rp + 1], in_=posm,
                                axis=mybir.AxisListType.X, op=OP.max)
    qidx_i = persist.tile([P, NXT], I32)
    tmpq = small.tile([P, NXT], F32, tag="tmpq")
    nc.vector.tensor_scalar(out=tmpq, in0=qposf, scalar1=-1.0, scalar2=0.0,
                            op0=OP.add, op1=OP.max)
    nc.vector.tensor_tensor(out=tmpq, in0=tmpq, in1=c_rb, op=OP.add)
    nc.vector.tensor_copy(out=qidx_i, in_=tmpq)
    pen_tok = persist.tile([P, NXT], F32)
    nc.vector.tensor_scalar(out=pen_tok, in0=presentf, scalar1=-1.0, scalar2=1e9,
                            op0=OP.add, op1=OP.mult)

    # ---- x0 gather (token-major fp32 master copy of x) ----
    x_f = [persist.tile([P, D], F32, tag=f"x{j}", name=f"x{j}")
           for j in range(NXT)]
    for j in range(NXT):
        nc.gpsimd.indirect_dma_start(
            out=x_f[j][:], out_offset=None, in_=io["seq"][:],
            in_offset=bass.IndirectOffsetOnAxis(ap=qidx_i[:, j:j + 1], axis=0))

    # ---- per-layer persistent buffers ----
    kT = persist.tile([P, NT], BF16)
    # v[cc]: (128, 4, 33) = [v_h | 1] per head (ones column -> denominators)
    v_sb = [persist.tile([P, H, HD + 1], BF16, tag=f"v{cc}", name=f"v{cc}")
            for cc in range(NTC)]
    for cc in range(NTC):
        nc.vector.memset(v_sb[cc][:, :, HD:HD + 1], 1.0)
    xnT = persist.tile([P, NX], BF16)
    q_sb = persist.tile([P, NX], BF16)
    qb2 = [persist.tile([P, H * C], BF16, tag=f"qb{i}", name=f"qb{i}")
           for i in range(2)]
    for i in range(2):
        nc.vector.memset(qb2[i], 0.0)

    def ln_to(dst_T):
        for g4 in range(NXT // 4):
            mvb = small.tile([P, 4, 2], F32, tag="mvb")
            for j in range(4):
                st = small.tile([P, 6], F32, tag="bnst")
                nc.vector.bn_stats(out=st, in_=x_f[4 * g4 + j])
                nc.vector.bn_aggr(out=mvb[:, j, :], in_=st)
            nc.scalar.activation(out=mvb[:, :, 1:2], in_=mvb[:, :, 1:2],
                                 func=AF.Sqrt, bias=c_eps[:])
            nc.vector.reciprocal(out=mvb[:, :, 1:2], in_=mvb[:, :, 1:2])
            for j in range(4):
                zx = ld.tile([P, D], BF16, tag="zxtok")
                nc.vector.tensor_scalar(out=zx, in0=x_f[4 * g4 + j],
                                        scalar1=mvb[:, j, 0:1],
                                        scalar2=mvb[:, j, 1:2],
                                        op0=OP.subtract, op1=OP.mult)
                pe_transpose_b(dst_T[:, (4 * g4 + j) * 128:(4 * g4 + j + 1) * 128],
                               zx)

    for l in range(L):
        # ---- kT = Wk' @ z (feature-major), bias via ACT evict ----
        for nn in range(NT // 512):
            ps = psA.tile([P, 512], F32, tag="mm")
            nc.tensor.matmul(out=ps, lhsT=w["wkt"][l][:],
                             rhs=zT[:, nn * 512:(nn + 1) * 512],
                             start=True, stop=True)
            nc.scalar.activation(out=kT[:, nn * 512:(nn + 1) * 512], in_=ps,
                                 func=AF.Identity, bias=bias["bk"][l][:])
        # ---- v token-major, head-pair layout with ones columns ----
        for cc in range(NTC):
            ps = psA.tile([P, D], F32, tag="mm")
            nc.tensor.matmul(out=ps, lhsT=zT[:, cc * 128:(cc + 1) * 128],
                             rhs=w["wvt"][l][:], start=True, stop=True)
            nc.scalar.copy(
                out=v_sb[cc][:, :, 0:HD],
                in_=ps[:].rearrange("p (h c) -> p h c", h=H))
        # ---- q = Wq' @ LN(x) ----
        ln_to(xnT)
        for nn in range(NX // 512):
            ps = psA.tile([P, 512], F32, tag="mm")
            nc.tensor.matmul(out=ps, lhsT=w["wqt"][l][:],
                             rhs=xnT[:, nn * 512:(nn + 1) * 512],
                             start=True, stop=True)
            nc.scalar.activation(out=q_sb[:, nn * 512:(nn + 1) * 512], in_=ps,
                                 func=AF.Identity, bias=bias["bq"][l][:])

        # ---- attention; 8 rows (one 512-token slab) at a time ----
        for sl in range(NX // 512):
            ctx_tok = []
            for rp2 in range(4):           # row pairs within slab
                psc = psC.tile([P, H, HD + 1], F32, tag="ctx")
                for par in range(2):
                    r = 8 * sl + 2 * rp2 + par
                    qb = qb2[r % 2]
                    for h in range(H):
                        nc.gpsimd.tensor_copy(
                            out=qb[HD * h:HD * (h + 1), C * h:C * (h + 1)],
                            in_=q_sb[HD * h:HD * (h + 1), C * r:C * (r + 1)])
                    e_ch = []
                    for c in range(2):
                        cc = 2 * r + c
                        ps = psA.tile([P, H * C], F32, tag="mm")
                        nc.tensor.matmul(out=ps,
                                         lhsT=kT[:, cc * 128:(cc + 1) * 128],
                                         rhs=qb, start=True, stop=False)
                        nc.tensor.matmul(
                            out=ps,
                            lhsT=m_gt[r // 2][64 * (r % 2):64 * (r % 2) + 64,
                                              c * 128:(c + 1) * 128],
                            rhs=c_nbe[64 * (r % 2):64 * (r % 2) + 64, :],
                            start=False, stop=True)
                        E = epool.tile([P, H * C], BF16, tag="E")
                        nc.scalar.activation(out=E, in_=ps, func=AF.Exp,
                                             scale=float(SCALE_S),
                                             bias=c_nbig[:])
                        e_ch.append(E)
                    off = 64 * par
                    for h in range(H):
                        for c in range(2):
                            nc.tensor.matmul(
                                out=psc[off:off + 64, h, :],
                                lhsT=e_ch[c][:, C * h:C * (h + 1)],
                                rhs=v_sb[2 * r + c][:, h, :],
                                start=(h == 0 and c == 0),
                                stop=(h == H - 1 and c == 1))
                rd = small.tile([P, H, 1], F32, tag="rd")
                nc.vector.tensor_scalar(out=rd, in0=psc[:, :, HD:HD + 1],
                                        scalar1=1e-30, scalar2=None, op0=OP.add)
                nc.vector.reciprocal(out=rd, in_=rd)
                ct = ctokp.tile([P, D], BF16, tag="ctok")
                nc.vector.scalar_tensor_tensor(
                    out=ct[:].rearrange("p (h c) -> p h c", h=H),
                    in0=psc[:, :, 0:HD], scalar=1.0,
                    in1=rd.to_broadcast([P, H, HD]),
                    op0=OP.mult, op1=OP.mult)
                ctx_tok.append(ct)
            cT = ev.tile([P, 512], BF16, tag="cT")
            for k in range(4):
                pe_transpose_b(cT[:, k * 128:(k + 1) * 128], ctx_tok[k])
            ps = psA.tile([P, 512], F32, tag="mm")
            nc.tensor.matmul(out=ps, lhsT=w["wot"][l][:], rhs=cT,
                             start=True, stop=True)
            aoT = ev.tile([P, 512], BF16, tag="aoT")
            nc.scalar.activation(out=aoT, in_=ps, func=AF.Identity,
                                 bias=bias["bo"][l][:])
            for k in range(4):
                j = sl * 4 + k
                pt = psT.tile([P, P], BF16, tag="pt")
                nc.tensor.transpose(out=pt, in_=aoT[:, k * 128:(k + 1) * 128],
                                    identity=c_idb[:])
                nc.vector.tensor_tensor(out=x_f[j], in0=x_f[j], in1=pt,
                                        op=OP.add)

        # ---- FFN ----
        ln_to(xnT)
        for nn in range(NX // 512):
            r1 = []
            for fc in range(4):
                ps = psA.tile([P, 512], F32, tag="mm")
                nc.tensor.matmul(out=ps,
                                 lhsT=w["w1t"][l][:, fc * 128:(fc + 1) * 128],
                                 rhs=xnT[:, nn * 512:(nn + 1) * 512],
                                 start=True, stop=True)
                r1t = ev.tile([P, 512], BF16, tag="r1")
                nc.scalar.activation(out=r1t, in_=ps, func=AF.Relu,
                                     bias=bias["b1_"][l][:, fc:fc + 1])
                r1.append(r1t)
            ps2 = psA.tile([P, 512], F32, tag="mm")
            for fc in range(4):
                nc.tensor.matmul(out=ps2,
                                 lhsT=w["w2t"][l][:, fc * 128:(fc + 1) * 128],
                                 rhs=r1[fc], start=(fc == 0), stop=(fc == 3))
            f2T = ev.tile([P, 512], BF16, tag="aoT")
            nc.scalar.activation(out=f2T, in_=ps2, func=AF.Identity,
                                 bias=bias["b2"][l][:])
            for k in range(4):
                j = nn * 4 + k
                pt = psT.tile([P, P], BF16, tag="pt")
                nc.tensor.transpose(out=pt, in_=f2T[:, k * 128:(k + 1) * 128],
                                    identity=c_idb[:])
                nc.vector.tensor_tensor(out=x_f[j], in0=x_f[j], in1=pt,
                                        op=OP.add)

    # ---- final stage (fp32) ----
    Lgr = persist.tile([P, R], F32)
    nc.vector.memset(Lgr, -1e9)
    scratch = small.tile([P, D], F32, tag="fsc")
    for r in range(R):
        off = 64 * (r % 2)
        tb = ld.tile([P, D], F32, tag="tgtbc")
        nc.gpsimd.dma_start(out=tb[off:off + 64, :], in_=bass.AP(
            tensor=io["tgt"].tensor, offset=r * D, ap=[[0, 64], [1, D]]))
        nc.vector.scalar_tensor_tensor(
            out=scratch[off:off + 64, :], in0=x_f[r // 2][off:off + 64, :],
            scalar=float(SCALE_L), in1=tb[off:off + 64, :],
            op0=OP.mult, op1=OP.mult)
        nc.vector.tensor_reduce(out=Lgr[off:off + 64, r:r + 1],
                                in_=scratch[off:off + 64, :],
                                axis=mybir.AxisListType.X, op=OP.add)
    for par in range(2):
        lp = Lgr[64 * par:64 * par + 64, :].rearrange("p (j two) -> p j two",
                                                      two=2)
        nc.vector.tensor_tensor(
            out=lp[:, :, par:par + 1], in0=lp[:, :, par:par + 1],
            in1=pen_tok[64 * par:64 * par + 64, :].rearrange(
                "p (j o) -> p j o", o=1),
            op=OP.add)
    psL = psX.tile([R, P], F32, tag="aux")
    nc.tensor.transpose(out=psL, in_=Lgr, identity=c_id[:])
    Erg = persist.tile([R, P], F32)
    den = small.tile([R, 1], F32, tag="den")
    nc.scalar.activation(out=Erg, in_=psL, func=AF.Exp, accum_out=den)
    nc.vector.reciprocal(out=den, in_=den)
    nc.vector.tensor_scalar(out=Erg, in0=Erg, scalar1=den, scalar2=None,
                            op0=OP.mult)
    psW = psX.tile([P, R], F32, tag="aux")
    nc.tensor.transpose(out=psW, in_=Erg, identity=c_id[0:R, 0:R])
    wT = persist.tile([P, R], F32)
    nc.vector.tensor_copy(out=wT, in_=psW)
    for j in range(NXT):
        psO = psX.tile([2, D], F32, tag="aux")
        nc.tensor.matmul(out=psO, lhsT=wT[:, 2 * j:2 * j + 2],
                         rhs=x_f[j][:], start=True, stop=True)
        o_sb = ev.tile([2, D], F32, tag="osb")
        nc.vector.tensor_copy(out=o_sb, in_=psO)
        nc.sync.dma_start(out=io["out"][2 * j:2 * j + 2, :], in_=o_sb)


# ---------------------------------------------------------------------------
# host side
# ---------------------------------------------------------------------------

_NC_CACHE = {}


def _get_nc():
    if "nc" not in _NC_CACHE:
        nc = bacc.Bacc("TRN2", target_bir_lowering=False, debug=False,
                       enable_asserts=False)
        _build(nc)
        nc.compile()
        _NC_CACHE["nc"] = nc
    return _NC_CACHE["nc"]


def _consts():
    p = np.arange(128)
    iota_g2 = (p % 64).astype(np.float32)[:, None]
    iota_t = np.tile((np.arange(T) + 1.0).astype(np.float32), (128, 1))
    col = np.arange(NXT)
    rowbase = (256.0 * (2 * col[None, :] + p[:, None] // 64)).astype(np.float32)
    ident = np.eye(128, dtype=np.float32)
    identb = np.eye(128, dtype=ml_dtypes.bfloat16)
    nbeye = np.zeros((2 * C, H * C), np.float32)
    for h in range(H):
        nbeye[:C, h * C:(h + 1) * C] = np.eye(C) * (BIG / SCALE_S)
    nbeye[C:] = nbeye[:C]
    return dict(iota_g2=iota_g2, iota_t=iota_t, rowbase=rowbase, ident=ident,
                identb=identb, nbeye=nbeye.astype(ml_dtypes.bfloat16))


def _prep_weights(inp):
    wqkv = np.asarray(inp["wqkv"], np.float32)
    bqkv = np.asarray(inp["bqkv"], np.float32)
    wo = np.asarray(inp["wo"], np.float32)
    bo = np.asarray(inp["bo"], np.float32)
    l1g = np.asarray(inp["ln1_g"], np.float32)
    l1b = np.asarray(inp["ln1_b"], np.float32)
    l2g = np.asarray(inp["ln2_g"], np.float32)
    l2b = np.asarray(inp["ln2_b"], np.float32)
    w1 = np.asarray(inp["w1"], np.float32)
    b1 = np.asarray(inp["b1"], np.float32)
    w2 = np.asarray(inp["w2"], np.float32)
    b2 = np.asarray(inp["b2"], np.float32)
    Wq, Wk, Wv = wqkv[:, :D], wqkv[:, D:2 * D], wqkv[:, 2 * D:]
    bq_, bk_, bv_ = bqkv[:, :D], bqkv[:, D:2 * D], bqkv[:, 2 * D:]
    bf = lambda x: np.ascontiguousarray(x.astype(ml_dtypes.bfloat16))
    f32 = lambda x: np.ascontiguousarray(x.astype(np.float32))
    m = {}
    for l in range(L):
        Wqp = Wq[l] * l1g[l][None, :]
        Wkp = Wk[l] * l1g[l][None, :]
        Wvp = Wv[l] * l1g[l][None, :]
        W1p = w1[l] * l2g[l][None, :]
        bqp = Wq[l] @ l1b[l] + bq_[l]
        bkp = Wk[l] @ l1b[l] + bk_[l]
        bvp = Wv[l] @ l1b[l] + bv_[l]
        b1p = w1[l] @ l2b[l] + b1[l]
        bop = wo[l] @ bvp + bo[l]          # v bias folded through wo
        # v layout on chip: head pairs [v0 | 1 | v1 | 1 | v2 | 1 | v3 | 1]
        m[f"wkt{l}"] = bf(Wkp.T)
        m[f"wvt{l}"] = bf(Wvp.T)
        m[f"wqt{l}"] = bf(Wqp.T)
        m[f"wot{l}"] = bf(wo[l].T)
        m[f"w1t{l}"] = bf(W1p.T)
        w2tl = np.empty((128, F), np.float32)
        for fc in range(4):
            w2tl[:, fc * 128:(fc + 1) * 128] = w2[l][:, fc * 128:(fc + 1) * 128].T
        m[f"w2t{l}"] = bf(w2tl)
        m[f"bk{l}"] = f32(bkp[:, None])
        m[f"bq{l}"] = f32(bqp[:, None])
        m[f"bo{l}"] = f32(bop[:, None])
        m[f"b2{l}"] = f32(b2[l][:, None])
        m[f"b1_{l}"] = f32(b1p.reshape(4, 128).T)
    return m


def kernel(**inputs):
    nc = _get_nc()
    wm = _prep_weights(inputs)
    cm = _consts()
    seq = np.asarray(inputs["sequence_item_emb"], np.float32)
    cat = np.asarray(inputs["sequence_cat_ids"])
    msk = np.asarray(inputs["sequence_mask"])
    tgt = np.asarray(inputs["target_item_emb"], np.float32)
    in_maps = []
    for i in range(NCORES):
        rs = slice(i * R, (i + 1) * R)
        im = dict(wm)
        im.update(cm)
        im["seq"] = np.ascontiguousarray(seq[rs].reshape(NT, D))
        im["catf"] = np.ascontiguousarray(cat[rs].astype(np.float32))
        im["maskf"] = np.ascontiguousarray(msk[rs].astype(np.float32))
        im["tgt"] = np.ascontiguousarray(tgt[rs])
        in_maps.append(im)
    res = run_bass_kernel_spmd(nc, in_maps, list(range(NCORES)))
    _NC_CACHE["last"] = res
    return np.concatenate([res.results[i]["out"] for i in range(NCORES)], axis=0)
